# revision 1
# baseline (speedup 1.0000x reference)
"""Trainium2 Bass kernel for nn_MoEDetector (moe_routing).

Strategy: data-parallel over batch B=8 -> one batch per NeuronCore.
Per-core program (all activations SBUF-resident, no DRAM spills):
  - router logits in fp32 (argmax-selection safe), group softmax ratios
  - GCN1 -> agg -> GCN2 -> agg -> residual+LayerNorm, matmuls in bf16
  - 7 expert matmuls (3 syn on LN output, 1 len + 3 sem on hs), exact
    gelu, per-token top-1 selection folded into per-token coefficients
  - cls projection
Host-side simplifications (exact):
  - the active len expert (short vs long) is fully determined by
    seq_lengths[b] (router masking forces the argmax), so each core
    gets only the active len weight and a 7-column router matrix
  - LN gain/bias folded into the syn expert weights
  - zero biases (the spec fills) are skipped; nonzero biases are
    supported via an extra K=1 rank-1 matmul accumulation step
"""

import numpy as np
import ml_dtypes
from contextlib import ExitStack

B, S, H = 8, 1024, 1536
THRESHOLD = 128
P = 128
ST = S // P          # 8 s-tiles
KT = H // P          # 12 h contraction tiles
TT = S // P          # 8 t-tiles for adjacency contraction
NCH = 512            # matmul moving free-dim chunk
NN = H // NCH        # 3 chunks of the H output dim
EPS = 1e-5

_BF16 = ml_dtypes.bfloat16

_prog_cache = {}


def _build_program(cfg, debug_taps=False):
    """cfg = (router_bias_nz, syn_bias_nz, len_bias_nz, sem_bias_nz, cls_bias_nz)"""
    import concourse.bass as bass
    import concourse.tile as tile
    from concourse import bacc, masks, mybir

    rb_nz, synb_nz, lenb_nz, semb_nz, clsb_nz = cfg
    f32 = mybir.dt.float32
    bf16 = mybir.dt.bfloat16
    AF = mybir.ActivationFunctionType
    ALU = mybir.AluOpType
    AX = mybir.AxisListType
    ts = bass.ts

    nc = bacc.Bacc("TRN2", target_bir_lowering=False, debug=False)

    # ---- DRAM I/O ----
    hs_d = nc.dram_tensor("hs", [S, H], f32, kind="ExternalInput").ap()
    adj_d = nc.dram_tensor("adj", [S, S], f32, kind="ExternalInput").ap()
    rw_d = nc.dram_tensor("rw", [H, 7], f32, kind="ExternalInput").ap()
    wg1_d = nc.dram_tensor("wg1", [H, H], bf16, kind="ExternalInput").ap()
    wg2_d = nc.dram_tensor("wg2", [H, H], bf16, kind="ExternalInput").ap()
    wsyn_d = nc.dram_tensor("wsyn", [3, H, H], bf16, kind="ExternalInput").ap()
    wlen_d = nc.dram_tensor("wlen", [H, H], bf16, kind="ExternalInput").ap()
    wsem_d = nc.dram_tensor("wsem", [3, H, H], bf16, kind="ExternalInput").ap()
    wcls_d = nc.dram_tensor("wcls", [H, 2], bf16, kind="ExternalInput").ap()
    br_d = nc.dram_tensor("br", [1, 7], f32, kind="ExternalInput").ap() if rb_nz else None
    bsyn_d = nc.dram_tensor("bsyn", [3, H], f32, kind="ExternalInput").ap() if synb_nz else None
    blen_d = nc.dram_tensor("blen", [1, H], f32, kind="ExternalInput").ap() if lenb_nz else None
    bsem_d = nc.dram_tensor("bsem", [3, H], f32, kind="ExternalInput").ap() if semb_nz else None
    bcls_d = nc.dram_tensor("bcls", [1, 2], f32, kind="ExternalInput").ap() if clsb_nz else None
    out_d = nc.dram_tensor("out", [S, 2], f32, kind="ExternalOutput").ap()
    taps = {}
    if debug_taps:
        for nm, shape, dt in [
            ("d_logit", [S, 7], f32), ("d_coef", [S, 7], f32),
            ("d_sup1", [S, H], bf16), ("d_x1T", [H, S], bf16),
            ("d_shared", [S, H], f32), ("d_fused", [S, H], f32),
            ("d_adjT", [S, S], bf16), ("d_hsT", [H, S], bf16),
        ]:
            taps[nm] = nc.dram_tensor(nm, shape, dt, kind="ExternalOutput").ap()

    hs_r = hs_d.rearrange("(a p) h -> p a h", p=P)
    adj_r = adj_d.rearrange("(a p) t -> p a t", p=P)
    rw_r = rw_d.rearrange("(k p) e -> p k e", p=P)
    wcls_r = wcls_d.rearrange("(k p) c -> p k c", p=P)
    out_r = out_d.rearrange("(a p) c -> p a c", p=P)

    with tile.TileContext(nc) as tc, ExitStack() as ctx:
        # ---- pools ----
        const = ctx.enter_context(tc.tile_pool(name="const", bufs=1))
        hspool = ctx.enter_context(tc.tile_pool(name="hspool", bufs=1))
        hstpool = ctx.enter_context(tc.tile_pool(name="hstpool", bufs=1))
        bigT = ctx.enter_context(tc.tile_pool(name="bigT", bufs=1))
        adjpool = ctx.enter_context(tc.tile_pool(name="adjpool", bufs=1))
        suppool = ctx.enter_context(tc.tile_pool(name="suppool", bufs=1))
        wpool = ctx.enter_context(tc.tile_pool(name="wpool", bufs=13))
        small = ctx.enter_context(tc.tile_pool(name="small", bufs=2))
        trans = ctx.enter_context(tc.tile_pool(name="trans", bufs=2))
        rowf32 = ctx.enter_context(tc.tile_pool(name="rowf32", bufs=2))
        acc = ctx.enter_context(tc.tile_pool(name="acc", bufs=4, space="PSUM"))
        tp = ctx.enter_context(tc.tile_pool(name="tp", bufs=3, space="PSUM"))
        spsum = ctx.enter_context(tc.tile_pool(name="spsum", bufs=1, space="PSUM"))

        # ---- constants ----
        id_f32 = const.tile([P, P], f32, tag="idf")
        masks.make_identity(nc, id_f32[:])
        id_bf = const.tile([P, P], bf16, tag="idb")
        masks.make_identity(nc, id_bf[:])
        rw_sb = const.tile([P, KT, 7], f32, tag="rw")
        nc.gpsimd.dma_start(rw_sb[:], rw_r)
        wcls_sb = const.tile([P, KT, 2], bf16, tag="wcls")
        nc.gpsimd.dma_start(wcls_sb[:], wcls_r)
        eps_t = const.tile([P, 1], f32, tag="eps")
        nc.vector.memset(eps_t[:], EPS)
        ones_row = None
        if any(x is not None for x in (br_d, bsyn_d, blen_d, bsem_d, bcls_d)):
            ones_row = const.tile([1, P], f32, tag="ones")
            nc.vector.memset(ones_row[:], 1.0)

        def bias_row(dram_ap, n, tag):
            t = const.tile([1, n], f32, tag=tag)
            nc.gpsimd.dma_start(t[:], dram_ap)
            return t

        br_sb = bias_row(br_d, 7, "br") if br_d is not None else None
        bsyn_sb = ([bias_row(bsyn_d[e : e + 1, :], H, f"bsyn{e}") for e in range(3)]
                   if bsyn_d is not None else None)
        blen_sb = bias_row(blen_d, H, "blen") if blen_d is not None else None
        bsem_sb = ([bias_row(bsem_d[e : e + 1, :], H, f"bsem{e}") for e in range(3)]
                   if bsem_d is not None else None)

        # ---- adjacency: degree-normalize rows, transpose to [t, s] bf16 ----
        # adj_bf shares the "sup" slot (it is dead before sup1 is written)
        adj_bf = suppool.tile([P, ST, S], bf16, tag="sup")
        adjT = adjpool.tile([P, TT, S], bf16, tag="adjT")
        for a in range(ST):
            araw = rowf32.tile([P, S], f32, tag="rowf32")
            nc.sync.dma_start(araw[:], adj_r[:, a, :])
            deg = small.tile([P, 1], f32, tag=f"deg{a}")
            nc.vector.tensor_reduce(deg[:], araw[:], axis=AX.X, op=ALU.add)
            nc.vector.tensor_scalar_max(deg[:], deg[:], 1e-9)
            nc.vector.reciprocal(deg[:], deg[:])
            nc.vector.tensor_scalar_mul(adj_bf[:, a, :], araw[:], deg[:])
        for a in range(ST):
            for t in range(TT):
                pt = tp.tile([P, P], bf16, tag="tp")
                nc.tensor.transpose(pt[:], adj_bf[:, a, ts(t, P)], id_bf[:])
                nc.any.tensor_copy(adjT[:, t, ts(a, P)], pt[:])

        # ---- hs load; transpose to hsT (fp32 pass feeds the router) ----
        hs_all = hspool.tile([P, ST, H], f32, tag="hs")
        for a in range(ST):
            nc.sync.dma_start(hs_all[:, a, :], hs_r[:, a, :])
        # Router: single-shot matmul per (k, m) into PSUM, accumulate over k
        # on the vector engine in SBUF. (start=True clears has_written for
        # the WHOLE bank, so interleaved per-m accumulation groups sharing
        # one bank corrupt each other — values survive, bits don't.)
        hsT = hstpool.tile([P, KT, S], bf16, tag="hsT")
        logit = small.tile([P, ST, 7], f32, tag="logit")
        nc.vector.memset(logit[:], 0.0)
        for k in range(KT):
            hTf = rowf32.tile([P, S], f32, tag="rowf32")
            for a in range(ST):
                pt = tp.tile([P, P], f32, tag="tp")
                nc.tensor.transpose(pt[:], hs_all[:, a, ts(k, P)], id_f32[:])
                nc.any.tensor_copy(hTf[:, ts(a, P)], pt[:])
                nc.any.tensor_copy(hsT[:, k, ts(a, P)], pt[:])
            rlog = spsum.tile([P, ST, 7], f32, tag="sp")
            for m in range(ST):
                nc.tensor.matmul(rlog[:, m, :], hTf[:, ts(m, P)], rw_sb[:, k, :],
                                 start=True, stop=True)
            nc.vector.tensor_add(logit[:], logit[:], rlog[:])
        if br_sb is not None:
            rlog = spsum.tile([P, ST, 7], f32, tag="sp")
            for m in range(ST):
                nc.tensor.matmul(rlog[:, m, :], ones_row[:], br_sb[:],
                                 start=True, stop=True)
            nc.vector.tensor_add(logit[:], logit[:], rlog[:])

        # ---- router math: group softmax ratios + top-1 coefficients ----
        # logits are O(1): exp() without max-subtraction is safe, and softmax
        # ratios are shift-invariant so this matches the reference exactly.
        e_sb = small.tile([P, ST, 7], f32, tag="esb")
        nc.scalar.activation(e_sb[:], logit[:], AF.Exp)
        syn_e = small.tile([P, ST], f32, tag="syn_e")
        nc.vector.tensor_reduce(syn_e[:], e_sb[:, :, 0:3], axis=AX.X, op=ALU.max)
        sem_e = small.tile([P, ST], f32, tag="sem_e")
        nc.vector.tensor_reduce(sem_e[:], e_sb[:, :, 4:7], axis=AX.X, op=ALU.max)
        rden = small.tile([P, ST], f32, tag="rden")
        nc.vector.tensor_add(rden[:], syn_e[:], sem_e[:])
        nc.vector.tensor_add(rden[:], rden[:], e_sb[:, :, 3])
        nc.vector.reciprocal(rden[:], rden[:])

        csyn = small.tile([P, ST, 3], f32, tag="csyn")
        csem = small.tile([P, ST, 3], f32, tag="csem")
        clen = small.tile([P, ST], f32, tag="clen")
        nc.vector.tensor_mul(clen[:], e_sb[:, :, 3], rden[:])

        def group_coefs(cout, base, w_e):
            """cout[:,:,e] = rden * w_e * mask_e; first-max argmax over logit
            columns base..base+2 (matches jnp.argmax tie-breaking)."""
            l0, l1, l2 = (logit[:, :, base + i] for i in range(3))
            s0 = small.tile([P, ST], f32, tag="s0")
            ge02 = small.tile([P, ST], f32, tag="ge02")
            nc.vector.tensor_tensor(out=s0[:], in0=l0, in1=l1, op=ALU.is_ge)
            nc.vector.tensor_tensor(out=ge02[:], in0=l0, in1=l2, op=ALU.is_ge)
            nc.vector.tensor_mul(s0[:], s0[:], ge02[:])
            s1 = small.tile([P, ST], f32, tag="s1")
            ge12 = small.tile([P, ST], f32, tag="ge12")
            nc.vector.tensor_tensor(out=ge12[:], in0=l1, in1=l2, op=ALU.is_ge)
            nc.vector.tensor_mul(s1[:], s0[:], ge12[:])
            nc.vector.tensor_tensor(out=s1[:], in0=ge12[:], in1=s1[:], op=ALU.subtract)
            s2 = small.tile([P, ST], f32, tag="s2")
            nc.vector.tensor_add(s2[:], s0[:], s1[:])
            nc.vector.tensor_scalar(out=s2[:], in0=s2[:], scalar1=-1.0, scalar2=1.0,
                                    op0=ALU.mult, op1=ALU.add)
            for e, sm in enumerate((s0, s1, s2)):
                nc.vector.tensor_mul(cout[:, :, e], sm[:], w_e)
                nc.vector.tensor_mul(cout[:, :, e], cout[:, :, e], rden[:])

        group_coefs(csyn, 0, syn_e[:])
        group_coefs(csem, 4, sem_e[:])

        if debug_taps:
            lr = taps["d_logit"].rearrange("(a p) e -> p a e", p=P)
            nc.gpsimd.dma_start(lr, logit[:])
            cr = taps["d_coef"].rearrange("(a p) e -> p a e", p=P)
            nc.gpsimd.dma_start(cr[:, :, 0:3], csyn[:])
            nc.gpsimd.dma_start(cr[:, :, 3:4], clen[:])
            nc.gpsimd.dma_start(cr[:, :, 4:7], csem[:])
            nc.gpsimd.dma_start(
                taps["d_hsT"].rearrange("(k p) s -> p k s", p=P), hsT[:])
            nc.gpsimd.dma_start(
                taps["d_adjT"].rearrange("(t p) s -> p t s", p=P), adjT[:])

        # ---- helpers for the dense [S,H] x [H,H] matmuls ----
        def load_wtiles(wdram):
            tiles = []
            for k in range(KT):
                wt = wpool.tile([P, H], bf16, tag="w")
                nc.sync.dma_start(wt[:], wdram[ts(k, P), :])
                tiles.append(wt)
            return tiles

        def weight_mm(lhsT_t, wtiles, evict, bias_sb=None):
            """evict(m, n, psum) with psum = (lhsT.T @ W + bias)[m-tile, n-chunk]"""
            for m in range(ST):
                for n in range(NN):
                    ps = acc.tile([P, NCH], f32, tag="acc")
                    for k in range(KT):
                        last = (k == KT - 1) and (bias_sb is None)
                        nc.tensor.matmul(ps[:], lhsT_t[:, k, ts(m, P)],
                                         wtiles[k][:, ts(n, NCH)],
                                         start=(k == 0), stop=last)
                    if bias_sb is not None:
                        nc.tensor.matmul(ps[:], ones_row[:],
                                         bias_sb[:, ts(n, NCH)],
                                         start=False, stop=True)
                    evict(m, n, ps)

        def transpose_into(dstT, src_of_k, m, ident):
            for k in range(KT):
                pt = tp.tile([P, P], ident.dtype, tag="tp")
                nc.tensor.transpose(pt[:], src_of_k(k), ident[:])
                nc.any.tensor_copy(dstT[:, k, ts(m, P)], pt[:])

        # ---- GCN layer 1 ----
        w_g1 = load_wtiles(wg1_d)
        sup1 = suppool.tile([P, TT, H], bf16, tag="sup")
        weight_mm(hsT, w_g1,
                  lambda m, n, ps: nc.vector.tensor_copy(sup1[:, m, ts(n, NCH)], ps[:]))
        if debug_taps:
            nc.gpsimd.dma_start(
                taps["d_sup1"].rearrange("(a p) h -> p a h", p=P), sup1[:])

        x1T = bigT.tile([P, KT, S], bf16, tag="bigT")
        for m in range(ST):
            x1row = trans.tile([P, H], bf16, tag="x1row")
            for n in range(NN):
                ps = acc.tile([P, NCH], f32, tag="acc")
                for t in range(TT):
                    nc.tensor.matmul(ps[:], adjT[:, t, ts(m, P)],
                                     sup1[:, t, ts(n, NCH)],
                                     start=(t == 0), stop=(t == TT - 1))
                nc.scalar.activation(x1row[:, ts(n, NCH)], ps[:], AF.Relu)
            transpose_into(x1T, lambda k: x1row[:, ts(k, P)], m, id_bf)

        if debug_taps:
            nc.gpsimd.dma_start(
                taps["d_x1T"].rearrange("(k p) s -> p k s", p=P), x1T[:])

        # ---- GCN layer 2 ----
        w_g2 = load_wtiles(wg2_d)
        sup2 = suppool.tile([P, TT, H], bf16, tag="sup")
        weight_mm(x1T, w_g2,
                  lambda m, n, ps: nc.vector.tensor_copy(sup2[:, m, ts(n, NCH)], ps[:]))

        # ---- agg2 + residual + LayerNorm (affine folded into syn weights) ----
        sharedT = bigT.tile([P, KT, S], bf16, tag="bigT")
        for m in range(ST):
            x2row = trans.tile([P, H], f32, tag="rowbig")
            for n in range(NN):
                ps = acc.tile([P, NCH], f32, tag="acc")
                for t in range(TT):
                    nc.tensor.matmul(ps[:], adjT[:, t, ts(m, P)],
                                     sup2[:, t, ts(n, NCH)],
                                     start=(t == 0), stop=(t == TT - 1))
                nc.scalar.activation(x2row[:, ts(n, NCH)], ps[:], AF.Relu)
            nc.vector.tensor_add(hs_all[:, m, :], hs_all[:, m, :], x2row[:])
            stats = small.tile([P, NN, 6], f32, tag="stats")
            for c in range(NN):
                nc.vector.bn_stats(stats[:, c, :], hs_all[:, m, ts(c, NCH)])
            mv = small.tile([P, 2], f32, tag="mv")
            nc.vector.bn_aggr(mv[:], stats[:])
            rstd = small.tile([P, 1], f32, tag="rstd")
            nc.scalar.activation(rstd[:], mv[:, 1:2], AF.Sqrt, bias=eps_t[:])
            nc.vector.reciprocal(rstd[:], rstd[:])
            nc.vector.tensor_scalar(out=hs_all[:, m, :], in0=hs_all[:, m, :],
                                    scalar1=mv[:, 0:1], scalar2=rstd[:],
                                    op0=ALU.subtract, op1=ALU.mult)
            transpose_into(sharedT, lambda k: hs_all[:, m, ts(k, P)], m, id_f32)

        if debug_taps:
            nc.gpsimd.dma_start(
                taps["d_shared"].rearrange("(a p) h -> p a h", p=P), hs_all[:])

        # ---- experts: gelu + weighted top-1 accumulation into hs_all ----
        experts = [(wsyn_d[e], sharedT, csyn[:, :, e],
                    bsyn_sb[e] if bsyn_sb else None) for e in range(3)]
        experts.append((wlen_d, hsT, clen[:, :], blen_sb))
        experts += [(wsem_d[e], hsT, csem[:, :, e],
                     bsem_sb[e] if bsem_sb else None) for e in range(3)]

        for ei, (wdram, lhsT_t, coef, bias_sb) in enumerate(experts):
            wt = load_wtiles(wdram)

            def evict(m, n, ps, ei=ei, coef=coef):
                g = trans.tile([P, NCH], f32, tag="rowbig")
                nc.scalar.activation(g[:], ps[:], AF.Gelu)
                dst = hs_all[:, m, ts(n, NCH)]
                if ei == 0:
                    nc.vector.tensor_scalar_mul(dst, g[:], coef[:, m : m + 1])
                else:
                    nc.vector.scalar_tensor_tensor(
                        out=dst, in0=g[:], scalar=coef[:, m : m + 1], in1=dst,
                        op0=ALU.mult, op1=ALU.add)

            weight_mm(lhsT_t, wt, evict, bias_sb=bias_sb)

        if debug_taps:
            nc.gpsimd.dma_start(
                taps["d_fused"].rearrange("(a p) h -> p a h", p=P), hs_all[:])

        # ---- fusedT + cls projection ----
        bcls_sb = bias_row(bcls_d, 2, "bcls") if bcls_d is not None else None
        fusedT = bigT.tile([P, KT, S], bf16, tag="bigT")
        cps = spsum.tile([P, ST, 2], f32, tag="sp")
        out_sb = small.tile([P, ST, 2], f32, tag="outsb")
        for m in range(ST):
            transpose_into(fusedT, lambda k: hs_all[:, m, ts(k, P)], m, id_f32)
            for k in range(KT):
                last = (k == KT - 1) and (bcls_sb is None)
                nc.tensor.matmul(cps[:, m, :], fusedT[:, k, ts(m, P)],
                                 wcls_sb[:, k, :], start=(k == 0), stop=last)
            if bcls_sb is not None:
                nc.tensor.matmul(cps[:, m, :], ones_row[:], bcls_sb[:],
                                 start=False, stop=True)
            nc.any.tensor_copy(out_sb[:, m, :], cps[:, m, :])
        nc.gpsimd.dma_start(out_r, out_sb[:])

    nc.compile()
    return nc


def _get_program(cfg):
    if cfg not in _prog_cache:
        _prog_cache[cfg] = _build_program(cfg)
    return _prog_cache[cfg]


def kernel(**inputs):
    from concourse import bass_utils

    hs = np.asarray(inputs["hidden_states"], dtype=np.float32)
    adj = np.asarray(inputs["adj_matrix"], dtype=np.float32)
    seq_lengths = np.asarray(inputs["seq_lengths"])
    router_w = np.asarray(inputs["router_w"], dtype=np.float32)
    router_b = np.asarray(inputs["router_b"], dtype=np.float32)
    gcn1_w = np.asarray(inputs["gcn1_w"], dtype=np.float32)
    gcn2_w = np.asarray(inputs["gcn2_w"], dtype=np.float32)
    ln_g = np.asarray(inputs["ln_g"], dtype=np.float32)
    ln_b = np.asarray(inputs["ln_b"], dtype=np.float32)
    syn_w = np.asarray(inputs["syn_w"], dtype=np.float32)
    syn_b = np.asarray(inputs["syn_b"], dtype=np.float32)
    len_short_w = np.asarray(inputs["len_short_w"], dtype=np.float32)
    len_short_b = np.asarray(inputs["len_short_b"], dtype=np.float32)
    len_long_w = np.asarray(inputs["len_long_w"], dtype=np.float32)
    len_long_b = np.asarray(inputs["len_long_b"], dtype=np.float32)
    sem_w = np.asarray(inputs["sem_w"], dtype=np.float32)
    sem_b = np.asarray(inputs["sem_b"], dtype=np.float32)
    cls_w = np.asarray(inputs["cls_w"], dtype=np.float32)
    cls_b = np.asarray(inputs["cls_b"], dtype=np.float32)

    # fold LN affine into syn expert weights: (x*g + b) @ W = x @ (g[:,None]*W) + b@W
    syn_w_f = (ln_g[None, :, None] * syn_w).astype(np.float32)
    syn_b_f = (syn_b + np.einsum("h,ehd->ed", ln_b, syn_w)).astype(np.float32)

    is_short = seq_lengths <= THRESHOLD

    cfg = (
        bool(np.any(router_b != 0)),
        bool(np.any(syn_b_f != 0)),
        bool(np.any(len_short_b != 0) or np.any(len_long_b != 0)),
        bool(np.any(sem_b != 0)),
        bool(np.any(cls_b != 0)),
    )
    nc = _get_program(cfg)

    wg1 = gcn1_w.astype(_BF16)
    wg2 = gcn2_w.astype(_BF16)
    wsyn = syn_w_f.astype(_BF16)
    wlen_s = len_short_w.astype(_BF16)
    wlen_l = len_long_w.astype(_BF16)
    wsem = sem_w.astype(_BF16)
    wcls = cls_w.astype(_BF16)

    in_maps = []
    for b in range(B):
        lencol = 3 if is_short[b] else 4
        rw7 = np.ascontiguousarray(np.concatenate(
            [router_w[:, 0:3], router_w[:, lencol : lencol + 1], router_w[:, 5:8]],
            axis=1, dtype=np.float32))
        m = {
            "hs": np.ascontiguousarray(hs[b]),
            "adj": np.ascontiguousarray(adj[b]),
            "rw": rw7,
            "wg1": wg1, "wg2": wg2, "wsyn": wsyn,
            "wlen": wlen_s if is_short[b] else wlen_l,
            "wsem": wsem, "wcls": wcls,
        }
        if cfg[0]:
            br7 = np.concatenate(
                [router_b[0:3], router_b[lencol : lencol + 1], router_b[5:8]])
            m["br"] = br7.reshape(1, 7).astype(np.float32)
        if cfg[1]:
            m["bsyn"] = syn_b_f
        if cfg[2]:
            m["blen"] = (len_short_b if is_short[b]
                         else len_long_b).reshape(1, H).astype(np.float32)
        if cfg[3]:
            m["bsem"] = sem_b.astype(np.float32)
        if cfg[4]:
            m["bcls"] = cls_b.reshape(1, 2).astype(np.float32)
        in_maps.append(m)

    try:
        res = bass_utils.run_bass_kernel_spmd(nc, in_maps, core_ids=list(range(B)))
    except Exception:
        # transient device wedge (NRT_EXEC_UNIT_UNRECOVERABLE) clears on retry
        res = bass_utils.run_bass_kernel_spmd(nc, in_maps, core_ids=list(range(B)))
    globals()["_last_results"] = res
    out = np.stack([res.results[b]["out"] for b in range(B)]).astype(np.float32)
    return out



# revision 9
# speedup vs baseline: 1.2162x; 1.2162x over previous
"""Trainium2 Bass kernel for nn_MoEDetector (moe_routing).

Strategy: data-parallel over batch B=8 -> one batch per NeuronCore.
Per-core program (all activations SBUF-resident, no DRAM spills):
  - router logits in fp32 (argmax-selection safe), group softmax ratios
  - GCN1 -> agg -> GCN2 -> agg -> residual+LayerNorm: all four GEMMs in
    RAW fp8e4 (e4m3) with DoubleRow perf mode (2 k-subtiles / matmul,
    0.5 cycles/row).  LayerNorm washes out the ~3% fp8 noise (validated
    numerically: end-to-end 3.5e-3 vs 3.9e-3 for all-bf16).
  - 7 expert GEMMs in SPLIT-3 fp8: x ~ xh+xl, W ~ wh+wl (each e4m3),
    psum = xh@wh + xl@wh + xh@wl via 3 DoubleRow matmuls per 256-deep
    k-pair = 1.5 cycles/row vs bf16's 2.0, and MORE accurate than bf16
    (weight/act recon ~0.1%).  Exact gelu, per-token top-1 selection
    folded into per-token coefficients.
  - cls projection in bf16.
Scaling scheme (PSUM accumulates at a power-of-2 multiple of the true
value; the eviction activation rescales):
  - weight GEMMs: W stored as e4m3(64*W) [+ residual e4m3(64W - wh)],
    activations stored at scale 1 (values O(1)); evict scale 1/64.
  - adjacency: A=adj/deg stored as e4m3(256*A); agg evict scale 1/256.
Host-side simplifications (exact):
  - the active len expert (short vs long) is fully determined by
    seq_lengths[b] (router masking forces the argmax), so each core
    gets only the active len weight and a 7-column router matrix
  - LN gain/bias folded into the syn expert weights
  - zero biases (the spec fills) are skipped; nonzero biases are
    supported via K=1 rank-1 bf16 matmul accumulation steps (scaled 64x
    to match the fp8 psum scale).
"""

import numpy as np
import ml_dtypes
from contextlib import ExitStack

B, S, H = 8, 1024, 1536
THRESHOLD = 128
P = 128
ST = S // P          # 8 s-tiles
KT = H // P          # 12 h contraction tiles
KP = KT // 2         # 6 DoubleRow k-pairs
TT = S // P          # 8 t-tiles for adjacency contraction
TP2 = TT // 2        # 4 DoubleRow t-pairs
NCH = 512            # matmul moving free-dim chunk
NN = H // NCH        # 3 chunks of the H output dim
EPS = 1e-5
WSC = 64.0           # weight fp8 scale
ASC = 256.0          # adjacency fp8 scale

_BF16 = ml_dtypes.bfloat16
_E4 = ml_dtypes.float8_e4m3

_prog_cache = {}


def _build_program(cfg, debug_taps=False):
    """cfg = (router_bias_nz, syn_bias_nz, len_bias_nz, sem_bias_nz, cls_bias_nz)"""
    import concourse.bass as bass
    import concourse.tile as tile
    from concourse import bacc, masks, mybir

    rb_nz, synb_nz, lenb_nz, semb_nz, clsb_nz = cfg
    f32 = mybir.dt.float32
    bf16 = mybir.dt.bfloat16
    fp8 = mybir.dt.float8e4
    AF = mybir.ActivationFunctionType
    ALU = mybir.AluOpType
    AX = mybir.AxisListType
    DR = mybir.MatmulPerfMode.DoubleRow
    ts = bass.ts

    nc = bacc.Bacc("TRN2", target_bir_lowering=False, debug=False)

    # ---- DRAM I/O ----
    hs_d = nc.dram_tensor("hs", [S, H], f32, kind="ExternalInput").ap()
    adj_d = nc.dram_tensor("adj", [S, S], f32, kind="ExternalInput").ap()
    rw_d = nc.dram_tensor("rw", [H, 7], f32, kind="ExternalInput").ap()
    wg1_d = nc.dram_tensor("wg1", [H, H], fp8, kind="ExternalInput").ap()
    wg2_d = nc.dram_tensor("wg2", [H, H], fp8, kind="ExternalInput").ap()
    # expert weights: hi and lo fp8 splits, experts stacked [7, H, H]
    # order: syn0..2 (LN-folded), len, sem0..2
    weh_d = nc.dram_tensor("weh", [7, H, H], fp8, kind="ExternalInput").ap()
    wel_d = nc.dram_tensor("wel", [7, H, H], fp8, kind="ExternalInput").ap()
    wcls_d = nc.dram_tensor("wcls", [H, 2], bf16, kind="ExternalInput").ap()
    br_d = nc.dram_tensor("br", [1, 7], f32, kind="ExternalInput").ap() if rb_nz else None
    # expert biases (scaled by WSC), stacked like weh
    beh_d = (nc.dram_tensor("beh", [7, H], bf16, kind="ExternalInput").ap()
             if (synb_nz or lenb_nz or semb_nz) else None)
    bcls_d = nc.dram_tensor("bcls", [1, 2], f32, kind="ExternalInput").ap() if clsb_nz else None
    out_d = nc.dram_tensor("out", [S, 2], f32, kind="ExternalOutput").ap()

    hs_r = hs_d.rearrange("(a p) h -> p a h", p=P)
    adj_r = adj_d.rearrange("(a p) t -> p a t", p=P)
    rw_r = rw_d.rearrange("(k p) e -> p k e", p=P)
    wcls_r = wcls_d.rearrange("(k p) c -> p k c", p=P)
    out_r = out_d.rearrange("(a p) c -> p a c", p=P)

    with tile.TileContext(nc) as tc, ExitStack() as ctx:
        # ---- pools ----
        const = ctx.enter_context(tc.tile_pool(name="const", bufs=1))
        hspool = ctx.enter_context(tc.tile_pool(name="hspool", bufs=1))
        hsplit = ctx.enter_context(tc.tile_pool(name="hsplit", bufs=1))
        shsplit = ctx.enter_context(tc.tile_pool(name="shsplit", bufs=1))
        adjpool = ctx.enter_context(tc.tile_pool(name="adjpool", bufs=1))
        spool = ctx.enter_context(tc.tile_pool(name="spool", bufs=1))
        wpool = ctx.enter_context(tc.tile_pool(name="wpool", bufs=2))
        small = ctx.enter_context(tc.tile_pool(name="small", bufs=2))
        trans = ctx.enter_context(tc.tile_pool(name="trans", bufs=2))
        rowf32 = ctx.enter_context(tc.tile_pool(name="rowf32", bufs=2))
        acc = ctx.enter_context(tc.tile_pool(name="acc", bufs=4, space="PSUM"))
        tp = ctx.enter_context(tc.tile_pool(name="tp", bufs=3, space="PSUM"))
        spsum = ctx.enter_context(tc.tile_pool(name="spsum", bufs=1, space="PSUM"))

        # ---- constants ----
        id_f32 = const.tile([P, P], f32, tag="idf")
        masks.make_identity(nc, id_f32[:])
        id_bf = const.tile([P, P], bf16, tag="idb")
        nc.any.tensor_copy(id_bf[:], id_f32[:])
        rw_sb = const.tile([P, KT, 7], f32, tag="rw")
        nc.gpsimd.dma_start(rw_sb[:], rw_r)
        wcls_sb = const.tile([P, KT, 2], bf16, tag="wcls")
        nc.gpsimd.dma_start(wcls_sb[:], wcls_r)
        eps_t = const.tile([P, 1], f32, tag="eps")
        nc.vector.memset(eps_t[:], EPS)
        ones_row = None
        if any(x is not None for x in (br_d, beh_d, bcls_d)):
            ones_row = const.tile([1, P], f32, tag="ones")
            nc.vector.memset(ones_row[:], 1.0)
            ones_bf = const.tile([1, P], bf16, tag="onesb")
            nc.vector.memset(ones_bf[:], 1.0)

        br_sb = None
        if br_d is not None:
            br_sb = const.tile([1, 7], f32, tag="br")
            nc.gpsimd.dma_start(br_sb[:], br_d)
        beh_sb = None
        if beh_d is not None:
            beh_sb = const.tile([1, 7, H], bf16, tag="beh")
            nc.gpsimd.dma_start(beh_sb[:], beh_d.rearrange("e h -> 1 e h"))

        # ---- adjacency: degree-normalize rows (x256), f32-transpose, cast fp8 ----
        # (fp8 PE transposes are rejected by walrus; transpose f32, cast on copy)
        adjT = adjpool.tile([P, TT, S], fp8, tag="adjT")
        for a in range(ST):
            araw = rowf32.tile([P, S], f32, tag="rowf32")
            nc.sync.dma_start(araw[:], adj_r[:, a, :])
            deg = small.tile([P, 1], f32, tag=f"deg{a}")
            nc.vector.tensor_reduce(deg[:], araw[:], axis=AX.X, op=ALU.add)
            nc.vector.tensor_scalar_max(deg[:], deg[:], 1e-9)
            nc.vector.reciprocal(deg[:], deg[:])
            nc.vector.tensor_scalar(out=araw[:], in0=araw[:],
                                    scalar1=deg[:, 0:1], scalar2=ASC,
                                    op0=ALU.mult, op1=ALU.mult)
            for t in range(TT):
                pt = tp.tile([P, P], f32, tag="tp")
                nc.tensor.transpose(pt[:], araw[:, ts(t, P)], id_f32[:])
                nc.any.tensor_copy(adjT[:, t, ts(a, P)], pt[:])

        # ---- hs load; f32 transpose feeds router; fp8 split feeds GEMMs ----
        hs_all = hspool.tile([P, ST, H], f32, tag="hs")
        for a in range(ST):
            nc.sync.dma_start(hs_all[:, a, :], hs_r[:, a, :])
        hsTh = hsplit.tile([P, KT, S], fp8, tag="hsTh")
        hsTl = hsplit.tile([P, KT, S], fp8, tag="hsTl")
        logit = small.tile([P, ST, 7], f32, tag="logit")
        nc.vector.memset(logit[:], 0.0)
        for k in range(KT):
            hTf = rowf32.tile([P, S], f32, tag="rowf32")
            for a in range(ST):
                pt = tp.tile([P, P], f32, tag="tp")
                nc.tensor.transpose(pt[:], hs_all[:, a, ts(k, P)], id_f32[:])
                nc.any.tensor_copy(hTf[:, ts(a, P)], pt[:])
            nc.scalar.activation(hsTh[:, k, :], hTf[:], AF.Copy)
            nc.vector.tensor_tensor(out=hsTl[:, k, :], in0=hTf[:],
                                    in1=hsTh[:, k, :], op=ALU.subtract)
            rlog = spsum.tile([P, ST, 7], f32, tag="sp")
            for m in range(ST):
                nc.tensor.matmul(rlog[:, m, :], hTf[:, ts(m, P)], rw_sb[:, k, :],
                                 start=True, stop=True)
            nc.vector.tensor_add(logit[:], logit[:], rlog[:])
        if br_sb is not None:
            rlog = spsum.tile([P, ST, 7], f32, tag="sp")
            for m in range(ST):
                nc.tensor.matmul(rlog[:, m, :], ones_row[:], br_sb[:],
                                 start=True, stop=True)
            nc.vector.tensor_add(logit[:], logit[:], rlog[:])

        # ---- router math: group softmax ratios + top-1 coefficients ----
        # logits are O(1): exp() without max-subtraction is safe, and softmax
        # ratios are shift-invariant so this matches the reference exactly.
        e_sb = small.tile([P, ST, 7], f32, tag="esb")
        nc.scalar.activation(e_sb[:], logit[:], AF.Exp)
        syn_e = small.tile([P, ST], f32, tag="syn_e")
        nc.vector.tensor_reduce(syn_e[:], e_sb[:, :, 0:3], axis=AX.X, op=ALU.max)
        sem_e = small.tile([P, ST], f32, tag="sem_e")
        nc.vector.tensor_reduce(sem_e[:], e_sb[:, :, 4:7], axis=AX.X, op=ALU.max)
        rden = small.tile([P, ST], f32, tag="rden")
        nc.vector.tensor_add(rden[:], syn_e[:], sem_e[:])
        nc.vector.tensor_add(rden[:], rden[:], e_sb[:, :, 3])
        nc.vector.reciprocal(rden[:], rden[:])

        csyn = small.tile([P, ST, 3], f32, tag="csyn")
        csem = small.tile([P, ST, 3], f32, tag="csem")
        clen = small.tile([P, ST], f32, tag="clen")
        nc.vector.tensor_mul(clen[:], e_sb[:, :, 3], rden[:])

        def group_coefs(cout, base, w_e):
            """cout[:,:,e] = rden * w_e * mask_e; first-max argmax over logit
            columns base..base+2 (matches jnp.argmax tie-breaking)."""
            l0, l1, l2 = (logit[:, :, base + i] for i in range(3))
            s0 = small.tile([P, ST], f32, tag="s0")
            ge02 = small.tile([P, ST], f32, tag="ge02")
            nc.vector.tensor_tensor(out=s0[:], in0=l0, in1=l1, op=ALU.is_ge)
            nc.vector.tensor_tensor(out=ge02[:], in0=l0, in1=l2, op=ALU.is_ge)
            nc.vector.tensor_mul(s0[:], s0[:], ge02[:])
            s1 = small.tile([P, ST], f32, tag="s1")
            ge12 = small.tile([P, ST], f32, tag="ge12")
            nc.vector.tensor_tensor(out=ge12[:], in0=l1, in1=l2, op=ALU.is_ge)
            nc.vector.tensor_mul(s1[:], s0[:], ge12[:])
            nc.vector.tensor_tensor(out=s1[:], in0=ge12[:], in1=s1[:], op=ALU.subtract)
            s2 = small.tile([P, ST], f32, tag="s2")
            nc.vector.tensor_add(s2[:], s0[:], s1[:])
            nc.vector.tensor_scalar(out=s2[:], in0=s2[:], scalar1=-1.0, scalar2=1.0,
                                    op0=ALU.mult, op1=ALU.add)
            for e, sm in enumerate((s0, s1, s2)):
                nc.vector.tensor_mul(cout[:, :, e], sm[:], w_e)
                nc.vector.tensor_mul(cout[:, :, e], cout[:, :, e], rden[:])

        group_coefs(csyn, 0, syn_e[:])
        group_coefs(csem, 4, sem_e[:])

        # ---- fp8 GEMM helpers ----
        def load_w8(wdram, tag_hint, q=None):
            wt = wpool.tile([P, KT, H], fp8, tag="w")
            (q or nc.sync).dma_start(wt[:], wdram.rearrange("(k p) n -> p k n", p=P))
            return wt

        def raw_mm(lhsT, wt, evict):
            """psum = (lhsT.T @ W)[m-tile, n-chunk] via raw fp8 DoubleRow."""
            for m in range(ST):
                for n in range(NN):
                    ps = acc.tile([P, NCH], f32, tag="acc")
                    for k2 in range(KP):
                        nc.tensor.matmul(ps[:], lhsT[:, 2 * k2:2 * k2 + 2, ts(m, P)],
                                         wt[:, 2 * k2:2 * k2 + 2, ts(n, NCH)],
                                         start=(k2 == 0), stop=(k2 == KP - 1),
                                         perf_mode=DR)
                    evict(m, n, ps)

        def split3_mm(lhsTh, lhsTl, wh, wl, evict, bias_sb=None):
            """psum = 64*(x@W)[m,n] via 3-term split fp8 DoubleRow."""
            for m in range(ST):
                for n in range(NN):
                    ps = acc.tile([P, NCH], f32, tag="acc")
                    for k2 in range(KP):
                        nc.tensor.matmul(ps[:], lhsTh[:, 2 * k2:2 * k2 + 2, ts(m, P)],
                                         wh[:, 2 * k2:2 * k2 + 2, ts(n, NCH)],
                                         start=(k2 == 0), stop=False, perf_mode=DR)
                    for k2 in range(KP):
                        nc.tensor.matmul(ps[:], lhsTl[:, 2 * k2:2 * k2 + 2, ts(m, P)],
                                         wh[:, 2 * k2:2 * k2 + 2, ts(n, NCH)],
                                         start=False, stop=False, perf_mode=DR)
                    for k2 in range(KP):
                        last = (k2 == KP - 1) and (bias_sb is None)
                        nc.tensor.matmul(ps[:], lhsTh[:, 2 * k2:2 * k2 + 2, ts(m, P)],
                                         wl[:, 2 * k2:2 * k2 + 2, ts(n, NCH)],
                                         start=False, stop=last, perf_mode=DR)
                    if bias_sb is not None:
                        nc.tensor.matmul(ps[:], ones_bf[:], bias_sb[:, ts(n, NCH)],
                                         start=False, stop=True)
                    evict(m, n, ps)

        # ---- GCN layer 1 (raw fp8) ----
        wg1 = load_w8(wg1_d, "g1")
        sup1 = spool.tile([P, ST, H], fp8, tag="slotA")
        raw_mm(hsTh, wg1,
               lambda m, n, ps: nc.scalar.activation(
                   sup1[:, m, ts(n, NCH)], ps[:], AF.Copy, scale=1.0 / WSC))

        # ---- agg1 + relu -> x1 (bf16 rows), transpose, cast fp8 for GCN2 ----
        x1T = spool.tile([P, KT, S], fp8, tag="slotB")
        for m in range(ST):
            x1row = trans.tile([P, H], bf16, tag="rowbf")
            for n in range(NN):
                ps = acc.tile([P, NCH], f32, tag="acc")
                for a2 in range(TP2):
                    nc.tensor.matmul(ps[:], adjT[:, 2 * a2:2 * a2 + 2, ts(m, P)],
                                     sup1[:, 2 * a2:2 * a2 + 2, ts(n, NCH)],
                                     start=(a2 == 0), stop=(a2 == TP2 - 1),
                                     perf_mode=DR)
                nc.scalar.activation(x1row[:, ts(n, NCH)], ps[:], AF.Relu,
                                     scale=1.0 / ASC)
            for k in range(KT):
                pt = tp.tile([P, P], bf16, tag="tp")
                nc.tensor.transpose(pt[:], x1row[:, ts(k, P)], id_bf[:])
                nc.any.tensor_copy(x1T[:, k, ts(m, P)], pt[:])

        # ---- GCN layer 2 (raw fp8) ----
        wg2 = load_w8(wg2_d, "g2")
        sup2 = spool.tile([P, ST, H], fp8, tag="slotA")
        raw_mm(x1T, wg2,
               lambda m, n, ps: nc.scalar.activation(
                   sup2[:, m, ts(n, NCH)], ps[:], AF.Copy, scale=1.0 / WSC))

        # ---- agg2 + relu + residual + LayerNorm -> sharedT fp8 splits ----
        shTh = shsplit.tile([P, KT, S], fp8, tag="shTh")
        shTl = shsplit.tile([P, KT, S], fp8, tag="shTl")
        for m in range(ST):
            x2row = trans.tile([P, H], f32, tag="rowbig")
            for n in range(NN):
                ps = acc.tile([P, NCH], f32, tag="acc")
                for a2 in range(TP2):
                    nc.tensor.matmul(ps[:], adjT[:, 2 * a2:2 * a2 + 2, ts(m, P)],
                                     sup2[:, 2 * a2:2 * a2 + 2, ts(n, NCH)],
                                     start=(a2 == 0), stop=(a2 == TP2 - 1),
                                     perf_mode=DR)
                nc.scalar.activation(x2row[:, ts(n, NCH)], ps[:], AF.Relu,
                                     scale=1.0 / ASC)
            nc.vector.tensor_add(hs_all[:, m, :], hs_all[:, m, :], x2row[:])
            stats = small.tile([P, NN, 6], f32, tag="stats")
            for c in range(NN):
                nc.vector.bn_stats(stats[:, c, :], hs_all[:, m, ts(c, NCH)])
            mv = small.tile([P, 2], f32, tag="mv")
            nc.vector.bn_aggr(mv[:], stats[:])
            rstd = small.tile([P, 1], f32, tag="rstd")
            nc.scalar.activation(rstd[:], mv[:, 1:2], AF.Sqrt, bias=eps_t[:])
            nc.vector.reciprocal(rstd[:], rstd[:])
            nc.vector.tensor_scalar(out=hs_all[:, m, :], in0=hs_all[:, m, :],
                                    scalar1=mv[:, 0:1], scalar2=rstd[:],
                                    op0=ALU.subtract, op1=ALU.mult)
            for k in range(KT):
                pt = tp.tile([P, P], f32, tag="tp")
                nc.tensor.transpose(pt[:], hs_all[:, m, ts(k, P)], id_f32[:])
                nc.scalar.activation(shTh[:, k, ts(m, P)], pt[:], AF.Copy)
                nc.vector.tensor_tensor(out=shTl[:, k, ts(m, P)], in0=pt[:],
                                        in1=shTh[:, k, ts(m, P)], op=ALU.subtract)

        # ---- experts: gelu + weighted top-1 accumulation into hs_all ----
        experts = [(weh_d[e], wel_d[e], shTh, shTl, csyn[:, :, e]) for e in range(3)]
        experts.append((weh_d[3], wel_d[3], hsTh, hsTl, clen[:, :]))
        experts += [(weh_d[4 + e], wel_d[4 + e], hsTh, hsTl, csem[:, :, e])
                    for e in range(3)]

        for ei, (whd, wld, lh, ll, coef) in enumerate(experts):
            wh = load_w8(whd, f"wh{ei}")
            wl = load_w8(wld, f"wl{ei}", q=nc.scalar)
            bias_sb = beh_sb[:, ei, :] if beh_sb is not None else None

            def evict(m, n, ps, ei=ei, coef=coef):
                g = trans.tile([P, NCH], f32, tag="rowbig")
                nc.scalar.activation(g[:], ps[:], AF.Gelu, scale=1.0 / WSC)
                dst = hs_all[:, m, ts(n, NCH)]
                if ei == 0:
                    nc.vector.tensor_scalar_mul(dst, g[:], coef[:, m:m + 1])
                else:
                    nc.vector.scalar_tensor_tensor(
                        out=dst, in0=g[:], scalar=coef[:, m:m + 1], in1=dst,
                        op0=ALU.mult, op1=ALU.add)

            split3_mm(lh, ll, wh, wl, evict, bias_sb=bias_sb)

        # ---- fusedT (bf16) + cls projection ----
        bcls_sb = None
        if bcls_d is not None:
            bcls_sb = const.tile([1, 2], f32, tag="bcls")
            nc.gpsimd.dma_start(bcls_sb[:], bcls_d)
        fusedT = spool.tile([P, KT, S], bf16, tag="slotB")
        cps = spsum.tile([P, ST, 2], f32, tag="sp")
        out_sb = small.tile([P, ST, 2], f32, tag="outsb")
        for m in range(ST):
            for k in range(KT):
                pt = tp.tile([P, P], bf16, tag="tp")
                fr = trans.tile([P, P], bf16, tag="fbrow")
                nc.any.tensor_copy(fr[:], hs_all[:, m, ts(k, P)])
                nc.tensor.transpose(pt[:], fr[:], id_bf[:])
                nc.any.tensor_copy(fusedT[:, k, ts(m, P)], pt[:])
            for k in range(KT):
                last = (k == KT - 1) and (bcls_sb is None)
                nc.tensor.matmul(cps[:, m, :], fusedT[:, k, ts(m, P)],
                                 wcls_sb[:, k, :], start=(k == 0), stop=last)
            if bcls_sb is not None:
                nc.tensor.matmul(cps[:, m, :], ones_row[:], bcls_sb[:],
                                 start=False, stop=True)
            nc.any.tensor_copy(out_sb[:, m, :], cps[:, m, :])
        nc.gpsimd.dma_start(out_r, out_sb[:])

    nc.compile()
    return nc


def _get_program(cfg):
    if cfg not in _prog_cache:
        _prog_cache[cfg] = _build_program(cfg)
    return _prog_cache[cfg]


def _split8(w, scale):
    """w -> (e4m3(scale*w), e4m3(scale*w - hi)) as fp8 arrays."""
    ws = (w * scale).astype(np.float32)
    hi = ws.astype(_E4)
    lo = (ws - hi.astype(np.float32)).astype(_E4)
    return hi, lo


def kernel(**inputs):
    from concourse import bass_utils

    hs = np.asarray(inputs["hidden_states"], dtype=np.float32)
    adj = np.asarray(inputs["adj_matrix"], dtype=np.float32)
    seq_lengths = np.asarray(inputs["seq_lengths"])
    router_w = np.asarray(inputs["router_w"], dtype=np.float32)
    router_b = np.asarray(inputs["router_b"], dtype=np.float32)
    gcn1_w = np.asarray(inputs["gcn1_w"], dtype=np.float32)
    gcn2_w = np.asarray(inputs["gcn2_w"], dtype=np.float32)
    ln_g = np.asarray(inputs["ln_g"], dtype=np.float32)
    ln_b = np.asarray(inputs["ln_b"], dtype=np.float32)
    syn_w = np.asarray(inputs["syn_w"], dtype=np.float32)
    syn_b = np.asarray(inputs["syn_b"], dtype=np.float32)
    len_short_w = np.asarray(inputs["len_short_w"], dtype=np.float32)
    len_short_b = np.asarray(inputs["len_short_b"], dtype=np.float32)
    len_long_w = np.asarray(inputs["len_long_w"], dtype=np.float32)
    len_long_b = np.asarray(inputs["len_long_b"], dtype=np.float32)
    sem_w = np.asarray(inputs["sem_w"], dtype=np.float32)
    sem_b = np.asarray(inputs["sem_b"], dtype=np.float32)
    cls_w = np.asarray(inputs["cls_w"], dtype=np.float32)
    cls_b = np.asarray(inputs["cls_b"], dtype=np.float32)

    # fold LN affine into syn expert weights: (x*g + b) @ W = x @ (g[:,None]*W) + b@W
    syn_w_f = (ln_g[None, :, None] * syn_w).astype(np.float32)
    syn_b_f = (syn_b + np.einsum("h,ehd->ed", ln_b, syn_w)).astype(np.float32)

    is_short = seq_lengths <= THRESHOLD

    cfg = (
        bool(np.any(router_b != 0)),
        bool(np.any(syn_b_f != 0)),
        bool(np.any(len_short_b != 0) or np.any(len_long_b != 0)),
        bool(np.any(sem_b != 0)),
        bool(np.any(cls_b != 0)),
    )
    nc = _get_program(cfg)

    g1h, _ = _split8(gcn1_w, WSC)
    g2h, _ = _split8(gcn2_w, WSC)
    synh = [None] * 3
    synl = [None] * 3
    for e in range(3):
        synh[e], synl[e] = _split8(syn_w_f[e], WSC)
    lsh, lsl = _split8(len_short_w, WSC)
    llh, lll = _split8(len_long_w, WSC)
    semh = [None] * 3
    seml = [None] * 3
    for e in range(3):
        semh[e], seml[e] = _split8(sem_w[e], WSC)
    wcls = cls_w.astype(_BF16)

    in_maps = []
    for b in range(B):
        lencol = 3 if is_short[b] else 4
        rw7 = np.ascontiguousarray(np.concatenate(
            [router_w[:, 0:3], router_w[:, lencol:lencol + 1], router_w[:, 5:8]],
            axis=1, dtype=np.float32))
        weh = np.stack(synh + [lsh if is_short[b] else llh] + semh)
        wel = np.stack(synl + [lsl if is_short[b] else lll] + seml)
        m = {
            "hs": np.ascontiguousarray(hs[b]),
            "adj": np.ascontiguousarray(adj[b]),
            "rw": rw7,
            "wg1": g1h, "wg2": g2h,
            "weh": weh, "wel": wel,
            "wcls": wcls,
        }
        if cfg[0]:
            br7 = np.concatenate(
                [router_b[0:3], router_b[lencol:lencol + 1], router_b[5:8]])
            m["br"] = br7.reshape(1, 7).astype(np.float32)
        if cfg[1] or cfg[2] or cfg[3]:
            lb = (len_short_b if is_short[b] else len_long_b)
            beh = np.stack([syn_b_f[0], syn_b_f[1], syn_b_f[2], lb,
                            sem_b[0], sem_b[1], sem_b[2]]) * WSC
            m["beh"] = beh.astype(_BF16)
        if cfg[4]:
            m["bcls"] = cls_b.reshape(1, 2).astype(np.float32)
        in_maps.append(m)

    try:
        res = bass_utils.run_bass_kernel_spmd(nc, in_maps, core_ids=list(range(B)))
    except Exception:
        # transient device wedge (NRT_EXEC_UNIT_UNRECOVERABLE) clears on retry
        res = bass_utils.run_bass_kernel_spmd(nc, in_maps, core_ids=list(range(B)))
    globals()["_last_results"] = res
    out = np.stack([res.results[b]["out"] for b in range(B)]).astype(np.float32)
    return out


# revision 14
# speedup vs baseline: 1.4964x; 1.2304x over previous
"""Trainium2 Bass kernel for nn_MoEDetector (moe_routing).

Strategy: data-parallel over batch B=8 -> one batch per NeuronCore.
Per-core program (all activations SBUF-resident, no DRAM spills):
  - router logits in fp32 (argmax-selection safe), group softmax ratios
  - GCN1 -> agg -> GCN2 -> agg -> residual+LayerNorm: all four GEMMs in
    RAW fp8e4 (e4m3) with DoubleRow perf mode (2 k-subtiles / matmul,
    0.5 cycles/row).  LayerNorm washes out the ~3% fp8 noise (validated
    numerically: end-to-end 3.5e-3 vs 3.9e-3 for all-bf16).
  - 7 expert GEMMs in SPLIT-3 fp8: x ~ xh+xl, W ~ wh+wl (each e4m3),
    psum = xh@wh + xl@wh + xh@wl via 3 DoubleRow matmuls per 256-deep
    k-pair = 1.5 cycles/row vs bf16's 2.0, and MORE accurate than bf16
    (weight/act recon ~0.1%).  Exact gelu, per-token top-1 selection
    folded into per-token coefficients.
  - cls projection in bf16.
Scaling scheme (PSUM accumulates at a power-of-2 multiple of the true
value; the eviction activation rescales):
  - weight GEMMs: W stored as e4m3(64*W) [+ residual e4m3(64W - wh)],
    activations stored at scale 1 (values O(1)); evict scale 1/64.
  - adjacency: A=adj/deg stored as e4m3(256*A); agg evict scale 1/256.
Host-side simplifications (exact):
  - the active len expert (short vs long) is fully determined by
    seq_lengths[b] (router masking forces the argmax), so each core
    gets only the active len weight and a 7-column router matrix
  - LN gain/bias folded into the syn expert weights
  - zero biases (the spec fills) are skipped; nonzero biases are
    supported via K=1 rank-1 bf16 matmul accumulation steps (scaled 64x
    to match the fp8 psum scale).
"""

import numpy as np
import ml_dtypes
from contextlib import ExitStack

B, S, H = 8, 1024, 1536
THRESHOLD = 128
P = 128
ST = S // P          # 8 s-tiles
KT = H // P          # 12 h contraction tiles
KP = KT // 2         # 6 DoubleRow k-pairs
TT = S // P          # 8 t-tiles for adjacency contraction
TP2 = TT // 2        # 4 DoubleRow t-pairs
NCH = 512            # matmul moving free-dim chunk
NN = H // NCH        # 3 chunks of the H output dim
EPS = 1e-5
WSC = 64.0           # weight fp8 scale
ASC = 256.0          # adjacency fp8 scale

_BF16 = ml_dtypes.bfloat16
_E4 = ml_dtypes.float8_e4m3

_prog_cache = {}


def _build_program(cfg, debug_taps=False):
    """cfg = (router_bias_nz, syn_bias_nz, len_bias_nz, sem_bias_nz, cls_bias_nz)"""
    import concourse.bass as bass
    import concourse.tile as tile
    from concourse import bacc, masks, mybir

    rb_nz, synb_nz, lenb_nz, semb_nz, clsb_nz = cfg
    f32 = mybir.dt.float32
    bf16 = mybir.dt.bfloat16
    fp8 = mybir.dt.float8e4
    AF = mybir.ActivationFunctionType
    ALU = mybir.AluOpType
    AX = mybir.AxisListType
    DR = mybir.MatmulPerfMode.DoubleRow
    ts = bass.ts

    nc = bacc.Bacc("TRN2", target_bir_lowering=False, debug=False)

    # ---- DRAM I/O ----
    hs_d = nc.dram_tensor("hs", [S, H], f32, kind="ExternalInput").ap()
    adj_d = nc.dram_tensor("adj", [S, S], f32, kind="ExternalInput").ap()
    rw_d = nc.dram_tensor("rw", [H, 7], f32, kind="ExternalInput").ap()
    wg1_d = nc.dram_tensor("wg1", [H, H], fp8, kind="ExternalInput").ap()
    wg2_d = nc.dram_tensor("wg2", [H, H], fp8, kind="ExternalInput").ap()
    # expert weights: hi and lo fp8 splits, experts stacked [7, H, H]
    # order: syn0..2 (LN-folded), len, sem0..2
    weh_d = nc.dram_tensor("weh", [7, H, H], fp8, kind="ExternalInput").ap()
    wel_d = nc.dram_tensor("wel", [7, H, H], fp8, kind="ExternalInput").ap()
    wcls_d = nc.dram_tensor("wcls", [H, 2], bf16, kind="ExternalInput").ap()
    br_d = nc.dram_tensor("br", [1, 7], f32, kind="ExternalInput").ap() if rb_nz else None
    # expert biases (scaled by WSC), stacked like weh
    beh_d = (nc.dram_tensor("beh", [7, H], bf16, kind="ExternalInput").ap()
             if (synb_nz or lenb_nz or semb_nz) else None)
    bcls_d = nc.dram_tensor("bcls", [1, 2], f32, kind="ExternalInput").ap() if clsb_nz else None
    out_d = nc.dram_tensor("out", [S, 2], f32, kind="ExternalOutput").ap()

    hs_r = hs_d.rearrange("(a p) h -> p a h", p=P)
    adj_r = adj_d.rearrange("(a p) t -> p a t", p=P)
    rw_r = rw_d.rearrange("(k p) e -> p k e", p=P)
    wcls_r = wcls_d.rearrange("(k p) c -> p k c", p=P)
    out_r = out_d.rearrange("(a p) c -> p a c", p=P)

    with tile.TileContext(nc) as tc, ExitStack() as ctx:
        # ---- pools ----
        const = ctx.enter_context(tc.tile_pool(name="const", bufs=1))
        hspool = ctx.enter_context(tc.tile_pool(name="hspool", bufs=1))
        hsplit = ctx.enter_context(tc.tile_pool(name="hsplit", bufs=1))
        shsplit = ctx.enter_context(tc.tile_pool(name="shsplit", bufs=1))
        adjpool = ctx.enter_context(tc.tile_pool(name="adjpool", bufs=1))
        spool = ctx.enter_context(tc.tile_pool(name="spool", bufs=1))
        wpool = ctx.enter_context(tc.tile_pool(name="wpool", bufs=6))
        small = ctx.enter_context(tc.tile_pool(name="small", bufs=2))
        trans = ctx.enter_context(tc.tile_pool(name="trans", bufs=2))
        rowf32 = ctx.enter_context(tc.tile_pool(name="rowf32", bufs=2))
        acc = ctx.enter_context(tc.tile_pool(name="acc", bufs=4, space="PSUM"))
        tp = ctx.enter_context(tc.tile_pool(name="tp", bufs=3, space="PSUM"))
        spsum = ctx.enter_context(tc.tile_pool(name="spsum", bufs=1, space="PSUM"))

        # ---- constants ----
        id_f32 = const.tile([P, P], f32, tag="idf")
        masks.make_identity(nc, id_f32[:])
        id_bf = const.tile([P, P], bf16, tag="idb")
        nc.any.tensor_copy(id_bf[:], id_f32[:])
        rw_sb = const.tile([P, KT, 7], f32, tag="rw")
        nc.gpsimd.dma_start(rw_sb[:], rw_r)
        wcls_sb = const.tile([P, KT, 2], bf16, tag="wcls")
        nc.gpsimd.dma_start(wcls_sb[:], wcls_r)
        eps_t = const.tile([P, 1], f32, tag="eps")
        nc.vector.memset(eps_t[:], EPS)
        ones_row = None
        if any(x is not None for x in (br_d, beh_d, bcls_d)):
            ones_row = const.tile([1, P], f32, tag="ones")
            nc.vector.memset(ones_row[:], 1.0)
            ones_bf = const.tile([1, P], bf16, tag="onesb")
            nc.vector.memset(ones_bf[:], 1.0)

        br_sb = None
        if br_d is not None:
            br_sb = const.tile([1, 7], f32, tag="br")
            nc.gpsimd.dma_start(br_sb[:], br_d)
        beh_sb = None
        if beh_d is not None:
            beh_sb = const.tile([1, 7, H], bf16, tag="beh")
            nc.gpsimd.dma_start(beh_sb[:], beh_d.rearrange("e h -> 1 e h"))

        # ---- batched transpose helper: 4 blocks -> one PSUM bank, 1 copy ----
        # (fp8 PE transposes are rejected by walrus; transpose f32/bf16 and
        # cast on the copy out.  4 transposes share one bank: the first has
        # start=True which zeroes the whole 2KB zero-region, later ones
        # accumulate onto zeroed bytes.)
        _cpeng = [nc.gpsimd, nc.vector]

        def transpose4(srcs, dst4, ident, dt, ceng):
            """srcs: list of <=4 [P,P] APs; dst4: [P,len,P] AP; casts on copy."""
            pt = tp.tile([P, 4, P], dt, tag="tp")
            nb = len(srcs)
            for j, s in enumerate(srcs):
                nc.tensor.matmul(pt[:, j, :], s, ident[:], start=(j == 0),
                                 stop=(j == nb - 1), is_transpose=True,
                                 skip_group_check=True)
            nc.any.tensor_copy(dst4, pt[:, 0:nb, :])

        # ---- adjacency: degree-normalize rows (x256), f32-transpose, cast fp8 ----
        adjT = adjpool.tile([P, TT, S], fp8, tag="adjT")
        for a in range(ST):
            araw = rowf32.tile([P, S], f32, tag="rowf32")
            nc.sync.dma_start(araw[:], adj_r[:, a, :])
            deg = small.tile([P, 1], f32, tag=f"deg{a}")
            nc.vector.tensor_reduce(deg[:], araw[:], axis=AX.X, op=ALU.add)
            nc.vector.tensor_scalar_max(deg[:], deg[:], 1e-9)
            nc.vector.reciprocal(deg[:], deg[:])
            nc.vector.tensor_scalar(out=araw[:], in0=araw[:],
                                    scalar1=deg[:, 0:1], scalar2=ASC,
                                    op0=ALU.mult, op1=ALU.mult)
            for t4 in range(TT // 4):
                transpose4([araw[:, ts(t, P)] for t in range(4 * t4, 4 * t4 + 4)],
                           adjT[:, 4 * t4:4 * t4 + 4, ts(a, P)], id_f32, f32,
                           _cpeng[t4 % 2])

        # ---- hs load; f32 transpose feeds router; fp8 split feeds GEMMs ----
        hs_all = hspool.tile([P, ST, H], f32, tag="hs")
        for a in range(ST):
            nc.sync.dma_start(hs_all[:, a, :], hs_r[:, a, :])
        hsTh = hsplit.tile([P, KT, S], fp8, tag="hsTh")
        hsTl = hsplit.tile([P, KT, S], fp8, tag="hsTl")
        logit = small.tile([P, ST, 7], f32, tag="logit")
        nc.vector.memset(logit[:], 0.0)
        for k in range(KT):
            hTf = rowf32.tile([P, S], f32, tag="rowf32")
            for g in range(2):
                transpose4([hs_all[:, a, ts(k, P)] for a in range(4 * g, 4 * g + 4)],
                           hTf[:, ts(g, 4 * P)], id_f32, f32, _cpeng[g])
            nc.scalar.activation(hsTh[:, k, :], hTf[:], AF.Copy)
            (nc.gpsimd if k % 2 else nc.vector).tensor_tensor(
                out=hsTl[:, k, :], in0=hTf[:], in1=hsTh[:, k, :], op=ALU.subtract)
            rlog = spsum.tile([P, ST, 7], f32, tag="sp")
            for m in range(ST):
                nc.tensor.matmul(rlog[:, m, :], hTf[:, ts(m, P)], rw_sb[:, k, :],
                                 start=True, stop=True)
            nc.vector.tensor_add(logit[:], logit[:], rlog[:])
        if br_sb is not None:
            rlog = spsum.tile([P, ST, 7], f32, tag="sp")
            for m in range(ST):
                nc.tensor.matmul(rlog[:, m, :], ones_row[:], br_sb[:],
                                 start=True, stop=True)
            nc.vector.tensor_add(logit[:], logit[:], rlog[:])

        # ---- router math: group softmax ratios + top-1 coefficients ----
        # logits are O(1): exp() without max-subtraction is safe, and softmax
        # ratios are shift-invariant so this matches the reference exactly.
        e_sb = small.tile([P, ST, 7], f32, tag="esb")
        nc.scalar.activation(e_sb[:], logit[:], AF.Exp)
        syn_e = small.tile([P, ST], f32, tag="syn_e")
        nc.vector.tensor_reduce(syn_e[:], e_sb[:, :, 0:3], axis=AX.X, op=ALU.max)
        sem_e = small.tile([P, ST], f32, tag="sem_e")
        nc.vector.tensor_reduce(sem_e[:], e_sb[:, :, 4:7], axis=AX.X, op=ALU.max)
        rden = small.tile([P, ST], f32, tag="rden")
        nc.vector.tensor_add(rden[:], syn_e[:], sem_e[:])
        nc.vector.tensor_add(rden[:], rden[:], e_sb[:, :, 3])
        nc.vector.reciprocal(rden[:], rden[:])

        csyn = small.tile([P, ST, 3], f32, tag="csyn")
        csem = small.tile([P, ST, 3], f32, tag="csem")
        clen = small.tile([P, ST], f32, tag="clen")
        nc.vector.tensor_mul(clen[:], e_sb[:, :, 3], rden[:])

        def group_coefs(cout, base, w_e):
            """cout[:,:,e] = rden * w_e * mask_e; first-max argmax over logit
            columns base..base+2 (matches jnp.argmax tie-breaking)."""
            l0, l1, l2 = (logit[:, :, base + i] for i in range(3))
            s0 = small.tile([P, ST], f32, tag="s0")
            ge02 = small.tile([P, ST], f32, tag="ge02")
            nc.vector.tensor_tensor(out=s0[:], in0=l0, in1=l1, op=ALU.is_ge)
            nc.vector.tensor_tensor(out=ge02[:], in0=l0, in1=l2, op=ALU.is_ge)
            nc.vector.tensor_mul(s0[:], s0[:], ge02[:])
            s1 = small.tile([P, ST], f32, tag="s1")
            ge12 = small.tile([P, ST], f32, tag="ge12")
            nc.vector.tensor_tensor(out=ge12[:], in0=l1, in1=l2, op=ALU.is_ge)
            nc.vector.tensor_mul(s1[:], s0[:], ge12[:])
            nc.vector.tensor_tensor(out=s1[:], in0=ge12[:], in1=s1[:], op=ALU.subtract)
            s2 = small.tile([P, ST], f32, tag="s2")
            nc.vector.tensor_add(s2[:], s0[:], s1[:])
            nc.vector.tensor_scalar(out=s2[:], in0=s2[:], scalar1=-1.0, scalar2=1.0,
                                    op0=ALU.mult, op1=ALU.add)
            for e, sm in enumerate((s0, s1, s2)):
                nc.vector.tensor_mul(cout[:, :, e], sm[:], w_e)
                nc.vector.tensor_mul(cout[:, :, e], cout[:, :, e], rden[:])

        group_coefs(csyn, 0, syn_e[:])
        group_coefs(csem, 4, sem_e[:])

        # ---- chunked fp8 GEMM helpers (stream weights by 512-col n-chunk) ----
        def load_wchunk(wr, n, q):
            wt = wpool.tile([P, KT, NCH], fp8, tag="w")
            q.dma_start(wt[:], wr[:, :, ts(n, NCH)])
            return wt

        def raw_mm(lhsT, wdram, evict):
            wr = wdram.rearrange("(k p) n -> p k n", p=P)
            for n in range(NN):
                wt = load_wchunk(wr, n, nc.sync if n % 2 else nc.scalar)
                for m in range(ST):
                    ps = acc.tile([P, NCH], f32, tag="acc")
                    for k2 in range(KP):
                        nc.tensor.matmul(ps[:], lhsT[:, 2 * k2:2 * k2 + 2, ts(m, P)],
                                         wt[:, 2 * k2:2 * k2 + 2, :],
                                         start=(k2 == 0), stop=(k2 == KP - 1),
                                         perf_mode=DR)
                    evict(m, n, ps)

        def split3_tile(ps, lh, ll, wh, wl, m, bias_sb, n):
            for k2 in range(KP):
                nc.tensor.matmul(ps[:], lh[:, 2 * k2:2 * k2 + 2, ts(m, P)],
                                 wh[:, 2 * k2:2 * k2 + 2, :],
                                 start=(k2 == 0), stop=False, perf_mode=DR)
            for k2 in range(KP):
                nc.tensor.matmul(ps[:], ll[:, 2 * k2:2 * k2 + 2, ts(m, P)],
                                 wh[:, 2 * k2:2 * k2 + 2, :],
                                 start=False, stop=False, perf_mode=DR)
            for k2 in range(KP):
                last = (k2 == KP - 1) and (bias_sb is None)
                nc.tensor.matmul(ps[:], lh[:, 2 * k2:2 * k2 + 2, ts(m, P)],
                                 wl[:, 2 * k2:2 * k2 + 2, :],
                                 start=False, stop=last, perf_mode=DR)
            if bias_sb is not None:
                nc.tensor.matmul(ps[:], ones_bf[:], bias_sb[:, ts(n, NCH)],
                                 start=False, stop=True)

        def split3_mm(lh, ll, whd, wld, evict, bias_sb=None):
            whr = whd.rearrange("(k p) n -> p k n", p=P)
            wlr = wld.rearrange("(k p) n -> p k n", p=P)
            for n in range(NN):
                wh = load_wchunk(whr, n, nc.sync)
                wl = load_wchunk(wlr, n, nc.scalar)
                for m in range(ST):
                    ps = acc.tile([P, NCH], f32, tag="acc")
                    split3_tile(ps, lh, ll, wh, wl, m, bias_sb, n)
                    evict(m, n, ps)

        def split3_mm_mouter(lh, ll, whd, wld, evict, bias_sb, after_m):
            whr = whd.rearrange("(k p) n -> p k n", p=P)
            wlr = wld.rearrange("(k p) n -> p k n", p=P)
            whs = [load_wchunk(whr, n, nc.sync) for n in range(NN)]
            wls = [load_wchunk(wlr, n, nc.scalar) for n in range(NN)]
            for m in range(ST):
                for n in range(NN):
                    ps = acc.tile([P, NCH], f32, tag="acc")
                    split3_tile(ps, lh, ll, whs[n], wls[n], m, bias_sb, n)
                    evict(m, n, ps)
                after_m(m)

        # ---- GCN layer 1 (raw fp8) ----
        sup1 = spool.tile([P, ST, H], fp8, tag="slotA")
        raw_mm(hsTh, wg1_d,
               lambda m, n, ps: nc.scalar.activation(
                   sup1[:, m, ts(n, NCH)], ps[:], AF.Copy, scale=1.0 / WSC))

        # ---- agg1 + relu -> x1 (bf16 rows), transpose, cast fp8 for GCN2 ----
        x1T = spool.tile([P, KT, S], fp8, tag="slotB")
        for m in range(ST):
            x1row = trans.tile([P, H], bf16, tag="rowbf")
            for n in range(NN):
                ps = acc.tile([P, NCH], f32, tag="acc")
                for a2 in range(TP2):
                    nc.tensor.matmul(ps[:], adjT[:, 2 * a2:2 * a2 + 2, ts(m, P)],
                                     sup1[:, 2 * a2:2 * a2 + 2, ts(n, NCH)],
                                     start=(a2 == 0), stop=(a2 == TP2 - 1),
                                     perf_mode=DR)
                nc.scalar.activation(x1row[:, ts(n, NCH)], ps[:], AF.Relu,
                                     scale=1.0 / ASC)
            for g in range(3):
                transpose4([x1row[:, ts(4 * g + j, P)] for j in range(4)],
                           x1T[:, 4 * g:4 * g + 4, ts(m, P)], id_bf, bf16,
                           _cpeng[(m + g) % 2])

        # ---- GCN layer 2 (raw fp8) ----
        sup2 = spool.tile([P, ST, H], fp8, tag="slotA")
        raw_mm(x1T, wg2_d,
               lambda m, n, ps: nc.scalar.activation(
                   sup2[:, m, ts(n, NCH)], ps[:], AF.Copy, scale=1.0 / WSC))

        # ---- agg2 + relu + residual + LayerNorm -> sharedT fp8 splits ----
        shTh = shsplit.tile([P, KT, S], fp8, tag="shTh")
        shTl = shsplit.tile([P, KT, S], fp8, tag="shTl")
        for m in range(ST):
            x2row = trans.tile([P, H], f32, tag="rowbig")
            for n in range(NN):
                ps = acc.tile([P, NCH], f32, tag="acc")
                for a2 in range(TP2):
                    nc.tensor.matmul(ps[:], adjT[:, 2 * a2:2 * a2 + 2, ts(m, P)],
                                     sup2[:, 2 * a2:2 * a2 + 2, ts(n, NCH)],
                                     start=(a2 == 0), stop=(a2 == TP2 - 1),
                                     perf_mode=DR)
                nc.scalar.activation(x2row[:, ts(n, NCH)], ps[:], AF.Relu,
                                     scale=1.0 / ASC)
            nc.vector.tensor_add(hs_all[:, m, :], hs_all[:, m, :], x2row[:])
            stats = small.tile([P, NN, 6], f32, tag="stats")
            for c in range(NN):
                nc.vector.bn_stats(stats[:, c, :], hs_all[:, m, ts(c, NCH)])
            mv = small.tile([P, 2], f32, tag="mv")
            nc.vector.bn_aggr(mv[:], stats[:])
            rstd = small.tile([P, 1], f32, tag="rstd")
            nc.scalar.activation(rstd[:], mv[:, 1:2], AF.Sqrt, bias=eps_t[:])
            nc.vector.reciprocal(rstd[:], rstd[:])
            nc.vector.tensor_scalar(out=hs_all[:, m, :], in0=hs_all[:, m, :],
                                    scalar1=mv[:, 0:1], scalar2=rstd[:],
                                    op0=ALU.subtract, op1=ALU.mult)
            for g in range(3):
                pt = tp.tile([P, 4, P], f32, tag="tp")
                for j in range(4):
                    nc.tensor.matmul(pt[:, j, :], hs_all[:, m, ts(4 * g + j, P)],
                                     id_f32[:], start=(j == 0), stop=(j == 3),
                                     is_transpose=True, skip_group_check=True)
                nc.scalar.activation(shTh[:, 4 * g:4 * g + 4, ts(m, P)], pt[:],
                                     AF.Copy)
                nc.vector.tensor_tensor(
                    out=shTl[:, 4 * g:4 * g + 4, ts(m, P)], in0=pt[:],
                    in1=shTh[:, 4 * g:4 * g + 4, ts(m, P)], op=ALU.subtract)

        # ---- experts: gelu + weighted top-1 accumulation into hs_all ----
        experts = [(weh_d[e], wel_d[e], shTh, shTl, csyn[:, :, e]) for e in range(3)]
        experts.append((weh_d[3], wel_d[3], hsTh, hsTl, clen[:, :]))
        experts += [(weh_d[4 + e], wel_d[4 + e], hsTh, hsTl, csem[:, :, e])
                    for e in range(3)]

        bcls_sb = None
        if bcls_d is not None:
            bcls_sb = const.tile([1, 2], f32, tag="bcls")
            nc.gpsimd.dma_start(bcls_sb[:], bcls_d)
        fusedT = spool.tile([P, KT, S], bf16, tag="slotB")
        cps = spsum.tile([P, ST, 2], f32, tag="sp")
        out_sb = small.tile([P, ST, 2], f32, tag="outsb")

        def tail_for_m(m):
            fb = trans.tile([P, H], bf16, tag="rowbf")
            nc.gpsimd.tensor_copy(fb[:], hs_all[:, m, :])
            for g in range(3):
                transpose4([fb[:, ts(4 * g + j, P)] for j in range(4)],
                           fusedT[:, 4 * g:4 * g + 4, ts(m, P)], id_bf, bf16,
                           _cpeng[g % 2])
            for k in range(KT):
                last = (k == KT - 1) and (bcls_sb is None)
                nc.tensor.matmul(cps[:, m, :], fusedT[:, k, ts(m, P)],
                                 wcls_sb[:, k, :], start=(k == 0), stop=last)
            if bcls_sb is not None:
                nc.tensor.matmul(cps[:, m, :], ones_row[:], bcls_sb[:],
                                 start=False, stop=True)
            nc.vector.tensor_copy(out_sb[:, m, :], cps[:, m, :])

        for ei, (whd, wld, lh, ll, coef) in enumerate(experts):
            bias_sb = beh_sb[:, ei, :] if beh_sb is not None else None

            def evict(m, n, ps, ei=ei, coef=coef):
                g = trans.tile([P, NCH], f32, tag="rowbig")
                nc.scalar.activation(g[:], ps[:], AF.Gelu, scale=1.0 / WSC)
                dst = hs_all[:, m, ts(n, NCH)]
                if ei == 0:
                    nc.vector.tensor_scalar_mul(dst, g[:], coef[:, m:m + 1])
                else:
                    nc.vector.scalar_tensor_tensor(
                        out=dst, in0=g[:], scalar=coef[:, m:m + 1], in1=dst,
                        op0=ALU.mult, op1=ALU.add)

            if ei < 6:
                split3_mm(lh, ll, whd, wld, evict, bias_sb=bias_sb)
            else:
                split3_mm_mouter(lh, ll, whd, wld, evict, bias_sb, tail_for_m)
        nc.gpsimd.dma_start(out_r, out_sb[:])

    nc.compile()
    return nc


def _get_program(cfg):
    if cfg not in _prog_cache:
        _prog_cache[cfg] = _build_program(cfg)
    return _prog_cache[cfg]


def _split8(w, scale):
    """w -> (e4m3(scale*w), e4m3(scale*w - hi)) as fp8 arrays."""
    ws = (w * scale).astype(np.float32)
    hi = ws.astype(_E4)
    lo = (ws - hi.astype(np.float32)).astype(_E4)
    return hi, lo


def kernel(**inputs):
    from concourse import bass_utils

    hs = np.asarray(inputs["hidden_states"], dtype=np.float32)
    adj = np.asarray(inputs["adj_matrix"], dtype=np.float32)
    seq_lengths = np.asarray(inputs["seq_lengths"])
    router_w = np.asarray(inputs["router_w"], dtype=np.float32)
    router_b = np.asarray(inputs["router_b"], dtype=np.float32)
    gcn1_w = np.asarray(inputs["gcn1_w"], dtype=np.float32)
    gcn2_w = np.asarray(inputs["gcn2_w"], dtype=np.float32)
    ln_g = np.asarray(inputs["ln_g"], dtype=np.float32)
    ln_b = np.asarray(inputs["ln_b"], dtype=np.float32)
    syn_w = np.asarray(inputs["syn_w"], dtype=np.float32)
    syn_b = np.asarray(inputs["syn_b"], dtype=np.float32)
    len_short_w = np.asarray(inputs["len_short_w"], dtype=np.float32)
    len_short_b = np.asarray(inputs["len_short_b"], dtype=np.float32)
    len_long_w = np.asarray(inputs["len_long_w"], dtype=np.float32)
    len_long_b = np.asarray(inputs["len_long_b"], dtype=np.float32)
    sem_w = np.asarray(inputs["sem_w"], dtype=np.float32)
    sem_b = np.asarray(inputs["sem_b"], dtype=np.float32)
    cls_w = np.asarray(inputs["cls_w"], dtype=np.float32)
    cls_b = np.asarray(inputs["cls_b"], dtype=np.float32)

    # fold LN affine into syn expert weights: (x*g + b) @ W = x @ (g[:,None]*W) + b@W
    syn_w_f = (ln_g[None, :, None] * syn_w).astype(np.float32)
    syn_b_f = (syn_b + np.einsum("h,ehd->ed", ln_b, syn_w)).astype(np.float32)

    is_short = seq_lengths <= THRESHOLD

    cfg = (
        bool(np.any(router_b != 0)),
        bool(np.any(syn_b_f != 0)),
        bool(np.any(len_short_b != 0) or np.any(len_long_b != 0)),
        bool(np.any(sem_b != 0)),
        bool(np.any(cls_b != 0)),
    )
    nc = _get_program(cfg)

    g1h, _ = _split8(gcn1_w, WSC)
    g2h, _ = _split8(gcn2_w, WSC)
    synh = [None] * 3
    synl = [None] * 3
    for e in range(3):
        synh[e], synl[e] = _split8(syn_w_f[e], WSC)
    lsh, lsl = _split8(len_short_w, WSC)
    llh, lll = _split8(len_long_w, WSC)
    semh = [None] * 3
    seml = [None] * 3
    for e in range(3):
        semh[e], seml[e] = _split8(sem_w[e], WSC)
    wcls = cls_w.astype(_BF16)

    in_maps = []
    for b in range(B):
        lencol = 3 if is_short[b] else 4
        rw7 = np.ascontiguousarray(np.concatenate(
            [router_w[:, 0:3], router_w[:, lencol:lencol + 1], router_w[:, 5:8]],
            axis=1, dtype=np.float32))
        weh = np.stack(synh + [lsh if is_short[b] else llh] + semh)
        wel = np.stack(synl + [lsl if is_short[b] else lll] + seml)
        m = {
            "hs": np.ascontiguousarray(hs[b]),
            "adj": np.ascontiguousarray(adj[b]),
            "rw": rw7,
            "wg1": g1h, "wg2": g2h,
            "weh": weh, "wel": wel,
            "wcls": wcls,
        }
        if cfg[0]:
            br7 = np.concatenate(
                [router_b[0:3], router_b[lencol:lencol + 1], router_b[5:8]])
            m["br"] = br7.reshape(1, 7).astype(np.float32)
        if cfg[1] or cfg[2] or cfg[3]:
            lb = (len_short_b if is_short[b] else len_long_b)
            beh = np.stack([syn_b_f[0], syn_b_f[1], syn_b_f[2], lb,
                            sem_b[0], sem_b[1], sem_b[2]]) * WSC
            m["beh"] = beh.astype(_BF16)
        if cfg[4]:
            m["bcls"] = cls_b.reshape(1, 2).astype(np.float32)
        in_maps.append(m)

    try:
        res = bass_utils.run_bass_kernel_spmd(nc, in_maps, core_ids=list(range(B)))
    except Exception:
        # transient device wedge (NRT_EXEC_UNIT_UNRECOVERABLE) clears on retry
        res = bass_utils.run_bass_kernel_spmd(nc, in_maps, core_ids=list(range(B)))
    globals()["_last_results"] = res
    out = np.stack([res.results[b]["out"] for b in range(B)]).astype(np.float32)
    return out
